# revision 23
# baseline (speedup 1.0000x reference)
"""Trainium2 Bass kernel for MetalAcceleratedAttention.

Full inputs -> full output. Sharding: 8 cores = 2 batches x 4 head-groups
(4 heads each).  Each core computes QKV projection for its heads, causal
attention (scores kept transposed: S^T[j, i]), and a *partial* output
projection over its 256 channels.  The 4 partials per batch are summed on
the host at gather time (row-parallel W_proj, all-reduce-as-unshard).

Attention math per core (heads packed in pairs along partitions):
  qT, kT: [128, 2048]  (pair-packed: head A dims on partitions 0-63, B 64-127)
  v_aug:  16 x [128, 260]  (natural [j, dh] layout + ones column per head)
  S^T tile = kT_chunk.T @ qT_chunk  (PE row-tiling: both heads concurrently)
  E = exp(SCALE * S^T)  (no max subtraction: |scores| < ~6 for this problem)
  out_aug^T[c, i] += v_aug_chunk.T @ E_chunk  (row 64 = softmax denominator)
  attn_out^T = out_aug^T[0:64] * broadcast(1/den)
"""

import numpy as np

import concourse.bass as bass
import concourse.tile as tile
from concourse import bacc, mybir
from concourse.bass_utils import run_bass_kernel_spmd

B, L, D, H = 2, 2048, 1024, 16
DH = D // H
SCALE = DH**-0.5
N_CORES = 8
HPC = 4  # heads per core
NPAIR = 2  # head pairs per core
LCHUNK = 512
N_LC = L // LCHUNK  # 4
N_JT = L // 128  # 16
f32 = mybir.dt.float32
f32r = mybir.dt.float32r
F_EXP = mybir.ActivationFunctionType.Exp

_CACHE = {}


def _build_nc(passes=1):
    nc = bacc.Bacc(
        "TRN2", target_bir_lowering=False, debug=False, num_devices=N_CORES
    )
    xT = nc.declare_dram_parameter("xT", [D, L], f32, isOutput=False)
    wq = nc.declare_dram_parameter("wq", [D, 256], f32, isOutput=False)
    wk = nc.declare_dram_parameter("wk", [D, 256], f32, isOutput=False)
    wv = nc.declare_dram_parameter("wv", [D, 260], f32, isOutput=False)
    wp = nc.declare_dram_parameter("wp", [256, D], f32, isOutput=False)
    bq = nc.declare_dram_parameter("bq", [NPAIR, 128, 1], f32, isOutput=False)
    bk = nc.declare_dram_parameter("bk", [NPAIR, 128, 1], f32, isOutput=False)
    bv = nc.declare_dram_parameter("bv", [1, 260], f32, isOutput=False)
    bp = nc.declare_dram_parameter("bp", [1, D], f32, isOutput=False)
    masks = nc.declare_dram_parameter("masks", [4, 128, 512], f32, isOutput=False)
    ident = nc.declare_dram_parameter("ident", [128, 128], f32, isOutput=False)
    ones = nc.declare_dram_parameter("ones", [1, 128], f32, isOutput=False)
    out = nc.declare_dram_parameter("out", [L, D], f32, isOutput=True)

    with tile.TileContext(nc) as tc:
        _body(tc, xT, wq, wk, wv, wp, bq, bk, bv, bp, masks, ident, ones, out,
              passes=passes)
    nc.compile()
    return nc


def _body(tc, xT, wq, wk, wv, wp, bq, bk, bv, bp, masks, ident, ones, out, passes=1):
    nc = tc.nc
    with (
        tc.tile_pool(name="const", bufs=1) as const,
        tc.tile_pool(name="persist", bufs=1) as persist,
        tc.tile_pool(name="xt", bufs=2) as xt_pool,
        tc.tile_pool(name="epool", bufs=4) as e_pool,
        tc.tile_pool(name="rpool", bufs=2) as r_pool,
        tc.tile_pool(name="opool", bufs=3) as o_pool,
        tc.tile_pool(name="s_ps", bufs=2, space="PSUM") as s_ps,
        tc.tile_pool(name="av_ps", bufs=1, space="PSUM") as av_ps,
        tc.tile_pool(name="mm_ps", bufs=2, space="PSUM") as mm_ps,
    ):
        # ---- DMAs ordered so the first QKV chains' deps land first ----
        wq_sb = []
        xt0 = []
        wk_sb = []
        wv_sb = []
        for kc in range(8):
            t = const.tile([128, 256], f32r, tag=f"wq{kc}", name=f"wq{kc}")
            nc.sync.dma_start(out=t, in_=wq[128 * kc : 128 * kc + 128, :].bitcast(f32r))
            wq_sb.append(t)
            t = xt_pool.tile([128, 512], f32r, tag=f"x{kc}", name=f"x{kc}_0")
            nc.sync.dma_start(out=t, in_=xT[128 * kc : 128 * kc + 128, 0:512].bitcast(f32r))
            xt0.append(t)
            t = const.tile([128, 256], f32r, tag=f"wk{kc}", name=f"wk{kc}")
            nc.sync.dma_start(out=t, in_=wk[128 * kc : 128 * kc + 128, :].bitcast(f32r))
            wk_sb.append(t)
            t = const.tile([128, 260], f32r, tag=f"wv{kc}", name=f"wv{kc}")
            nc.sync.dma_start(out=t, in_=wv[128 * kc : 128 * kc + 128, :].bitcast(f32r))
            wv_sb.append(t)
        mask_sb = []
        for r in range(4):
            t = const.tile([128, 512], f32r, tag=f"mask{r}")
            nc.sync.dma_start(out=t, in_=masks[r].bitcast(f32r))
            mask_sb.append(t)
        ident_sb = const.tile([128, 128], f32r, tag="ident")
        nc.sync.dma_start(out=ident_sb, in_=ident[:, :].bitcast(f32r))
        bq_sb = []
        bk_sb = []
        for p in range(NPAIR):
            t = const.tile([128, 1], f32, tag=f"bq{p}")
            nc.sync.dma_start(out=t, in_=bq[p])
            bq_sb.append(t)
            t = const.tile([128, 1], f32, tag=f"bk{p}")
            nc.sync.dma_start(out=t, in_=bk[p])
            bk_sb.append(t)
        bv_sb = const.tile([1, 260], f32r, tag="bv")
        nc.sync.dma_start(out=bv_sb, in_=bv[:, :].bitcast(f32r))
        bp_sb = const.tile([1, D], f32r, tag="bp")
        nc.sync.dma_start(out=bp_sb, in_=bp[:, :].bitcast(f32r))
        ones_row = const.tile([1, 128], f32r, tag="ones")
        nc.sync.dma_start(out=ones_row, in_=ones[:, :].bitcast(f32r))
        wp_sb = []
        for cc in range(2):
            t = const.tile([128, D], f32r, tag=f"wp{cc}", name=f"wp{cc}")
            nc.sync.dma_start(out=t, in_=wp[128 * cc : 128 * cc + 128, :].bitcast(f32r))
            wp_sb.append(t)

        # broadcast b_proj to all partitions via rank-1 matmul (emitted lazily
        # before proj(0) so it does not block the PE pipeline head)
        bias_bc = persist.tile([128, D], f32, tag="bias_bc")

        def emit_bias_bc():
            for eh in range(2):
                ps = s_ps.tile([128, 512], f32, tag="s", name=f"bb{eh}")
                nc.tensor.matmul(
                    ps,
                    lhsT=ones_row,
                    rhs=bp_sb[:, 512 * eh : 512 * eh + 512],
                    start=True,
                    stop=True,
                )
                nc.vector.tensor_copy(bias_bc[:, 512 * eh : 512 * eh + 512], ps)

        # ---- persistent activations ----
        qT = [persist.tile([128, L], f32r, tag=f"qT{p}", name=f"qT{p}") for p in range(NPAIR)]
        kT = [persist.tile([128, L], f32r, tag=f"kT{p}", name=f"kT{p}") for p in range(NPAIR)]
        v_sb = [persist.tile([128, 260], f32r, tag=f"v{t}", name=f"v{t}") for t in range(N_JT)]
        aoT = [persist.tile([128, L], f32r, tag=f"aoT{p}", name=f"aoT{p}") for p in range(NPAIR)]

        _pc = [0]

        def proj(lc):
            # ---- partial output projection for one l-chunk ----
            _pc[0] += 1
            for lt in range(4):
                l0 = 512 * lc + 128 * lt
                for eh in range(2):
                    ps = mm_ps.tile([128, 512], f32, tag="mm", name=f"pj{_pc[0]}_{lc}_{lt}_{eh}")
                    for cc in range(2):
                        nc.tensor.matmul(
                            ps,
                            lhsT=aoT[cc][:, l0 : l0 + 128],
                            rhs=wp_sb[cc][:, 512 * eh : 512 * eh + 512],
                            start=(cc == 0),
                            stop=(cc == 1),
                        )
                    o = o_pool.tile([128, 512], f32, tag="o", name=f"o{_pc[0]}_{lc}_{lt}_{eh}")
                    nc.vector.tensor_add(o, ps, bias_bc[:, 512 * eh : 512 * eh + 512])
                    nc.sync.dma_start(
                        out=out[l0 : l0 + 128, 512 * eh : 512 * eh + 512], in_=o
                    )

        for rep in range(passes):
          for lc in range(N_LC):
            sl = slice(512 * lc, 512 * lc + 512)
            # ---- QKV for this l-chunk ----
            if lc == 0 and rep == 0:
                xt = xt0
            else:
                xt = []
                for kc in range(8):
                    t = xt_pool.tile([128, 512], f32r, tag=f"x{kc}", name=f"x{kc}_{rep}_{lc}")
                    nc.sync.dma_start(
                        out=t, in_=xT[128 * kc : 128 * kc + 128, sl].bitcast(f32r)
                    )
                    xt.append(t)
            for p in range(NPAIR):
                ps = mm_ps.tile([128, 512], f32, tag="mm")
                for kc in range(8):
                    nc.tensor.matmul(
                        ps,
                        lhsT=wq_sb[kc][:, 128 * p : 128 * p + 128],
                        rhs=xt[kc],
                        start=(kc == 0),
                        stop=(kc == 7),
                    )
                nc.vector.tensor_scalar_add(qT[p][:, sl], ps, bq_sb[p])
                ps = mm_ps.tile([128, 512], f32, tag="mm")
                for kc in range(8):
                    nc.tensor.matmul(
                        ps,
                        lhsT=wk_sb[kc][:, 128 * p : 128 * p + 128],
                        rhs=xt[kc],
                        start=(kc == 0),
                        stop=(kc == 7),
                    )
                nc.vector.tensor_scalar_add(kT[p][:, sl], ps, bk_sb[p])
            for ti in range(4):
                jt = 4 * lc + ti
                ps = mm_ps.tile([128, 512], f32, tag="mm")
                psv = ps[:, 0:260]
                for kc in range(8):
                    nc.tensor.matmul(
                        psv,
                        lhsT=xt[kc][:, 128 * ti : 128 * ti + 128],
                        rhs=wv_sb[kc],
                        start=(kc == 0),
                        stop=False,
                    )
                nc.tensor.matmul(psv, lhsT=ones_row, rhs=bv_sb, start=False, stop=True)
                nc.vector.tensor_copy(v_sb[jt], psv)

            # ---- projection for the previous l-chunk (pipelines with attention) ----
            if lc == 1 and rep == 0:
                emit_bias_bc()
            if lc > 0:
                proj(lc - 1)

            # ---- attention for i-chunk == lc ----
            n_j = 4 * (lc + 1)
            for p in range(NPAIR):
                avA = av_ps.tile([128, 512], f32, tag="avA")
                avB = av_ps.tile([128, 512], f32, tag="avB")
                for jt in range(n_j):
                    s = s_ps.tile([128, 1024], f32, tag="s")
                    diag = jt >= 4 * lc
                    if diag:
                        msk = mask_sb[jt - 4 * lc]
                        nc.tensor.matmul(
                            s[:, 0:512], lhsT=ident_sb, rhs=msk, start=True, stop=False
                        )
                        nc.tensor.matmul(
                            s[:, 512:1024], lhsT=ident_sb, rhs=msk, start=True, stop=False
                        )
                    nc.tensor.matmul(
                        s[:, 0:512],
                        lhsT=kT[p][0:64, 128 * jt : 128 * jt + 128],
                        rhs=qT[p][0:64, sl],
                        start=not diag,
                        stop=True,
                    )
                    nc.tensor.matmul(
                        s[:, 512:1024],
                        lhsT=kT[p][64:128, 128 * jt : 128 * jt + 128],
                        rhs=qT[p][64:128, sl],
                        start=not diag,
                        stop=True,
                    )
                    e = e_pool.tile([128, 1024], f32r, tag="e")
                    nc.scalar.activation(e, s, F_EXP, scale=SCALE)
                    nc.tensor.matmul(
                        avA[0:65, :],
                        lhsT=v_sb[jt][:, 130 * p : 130 * p + 65],
                        rhs=e[:, 0:512],
                        start=(jt == 0),
                        stop=(jt == n_j - 1),
                    )
                    nc.tensor.matmul(
                        avB[0:65, :],
                        lhsT=v_sb[jt][:, 130 * p + 65 : 130 * p + 130],
                        rhs=e[:, 512:1024],
                        start=(jt == 0),
                        stop=(jt == n_j - 1),
                    )
                rA = r_pool.tile([1, 512], f32r, tag="rA")
                rB = r_pool.tile([1, 512], f32r, tag="rB")
                with nc.allow_low_precision(reason="f32r is 4-byte fp32"):
                    nc.vector.reciprocal(rA, avA[64:65, :])
                    nc.vector.reciprocal(rB, avB[64:65, :])
                bcA = r_pool.tile([64, 512], f32r, tag="bcA")
                nc.gpsimd.partition_broadcast(bcA, rA, channels=64)
                bcB = r_pool.tile([64, 512], f32r, tag="bcB")
                nc.gpsimd.partition_broadcast(bcB, rB, channels=64)
                nc.vector.tensor_mul(aoT[p][0:64, sl], avA[0:64, :], bcA)
                nc.vector.tensor_mul(aoT[p][64:128, sl], avB[0:64, :], bcB)

        proj(N_LC - 1)


def _host_masks():
    m = np.zeros((4, 128, 512), dtype=np.float32)
    p = np.arange(128)[:, None]
    f = np.arange(512)[None, :]
    for r in range(4):
        m[r] = np.where(p + 128 * r <= f, 0.0, -2000.0).astype(np.float32)
    return m


def _shard_inputs(x, W_qkv, b_qkv, W_proj, b_proj):
    Wr = np.asarray(W_qkv, dtype=np.float32).reshape(D, 3, H, DH)
    br = np.asarray(b_qkv, dtype=np.float32).reshape(3, H, DH)
    masks = _host_masks()
    in_maps = []
    for c in range(N_CORES):
        b, g = divmod(c, 4)
        heads = slice(HPC * g, HPC * g + HPC)
        xTc = np.ascontiguousarray(np.asarray(x[b], dtype=np.float32).T)
        wq_c = np.ascontiguousarray(Wr[:, 0, heads, :].reshape(D, 256))
        wk_c = np.ascontiguousarray(Wr[:, 1, heads, :].reshape(D, 256))
        wv_c = np.zeros((D, 260), dtype=np.float32)
        bv_c = np.zeros((1, 260), dtype=np.float32)
        for hh in range(HPC):
            col = 130 * (hh // 2) + 65 * (hh % 2)
            wv_c[:, col : col + 64] = Wr[:, 2, HPC * g + hh, :]
            bv_c[0, col : col + 64] = br[2, HPC * g + hh, :]
            bv_c[0, col + 64] = 1.0
        bq_c = br[0, heads, :].reshape(NPAIR, 128, 1).astype(np.float32)
        bk_c = br[1, heads, :].reshape(NPAIR, 128, 1).astype(np.float32)
        wp_c = np.ascontiguousarray(
            np.asarray(W_proj, dtype=np.float32)[256 * g : 256 * g + 256, :]
        )
        bp_c = (
            np.asarray(b_proj, dtype=np.float32).reshape(1, D)
            if g == 0
            else np.zeros((1, D), dtype=np.float32)
        )
        in_maps.append(
            {
                "xT": xTc,
                "wq": wq_c,
                "wk": wk_c,
                "wv": wv_c,
                "wp": wp_c,
                "bq": np.ascontiguousarray(bq_c),
                "bk": np.ascontiguousarray(bk_c),
                "bv": bv_c,
                "bp": bp_c,
                "masks": masks,
                "ident": np.eye(128, dtype=np.float32),
                "ones": np.ones((1, 128), dtype=np.float32),
            }
        )
    return in_maps


def get_nc(passes=1):
    key = f"nc{passes}"
    if key not in _CACHE:
        _CACHE[key] = _build_nc(passes=passes)
    return _CACHE[key]


def _get_runner():
    """Persistent jitted shard_map executor (compiled once per process)."""
    if "runner" in _CACHE:
        return _CACHE["runner"]
    import jax
    from concourse import bass2jax
    from concourse.bass2jax import _bass_exec_p, install_neuronx_cc_hook
    from jax.sharding import Mesh, PartitionSpec
    from jax.experimental.shard_map import shard_map

    nc = get_nc()
    install_neuronx_cc_hook()
    in_names, out_names, out_avals, zero_outs = [], [], [], []
    pid = nc.partition_id_tensor.name if nc.partition_id_tensor else None
    for alloc in nc.m.functions[0].allocations:
        if not isinstance(alloc, mybir.MemoryLocationSet):
            continue
        name = alloc.memorylocations[0].name
        if alloc.kind == "ExternalInput":
            if name != pid:
                in_names.append(name)
        elif alloc.kind == "ExternalOutput":
            shape = list(alloc.tensor_shape)
            np_dt = mybir.dt.np(alloc.dtype)
            out_avals.append(jax.core.ShapedArray(shape, np_dt))
            out_names.append(name)
            zero_outs.append(np.zeros(shape, np_dt))
    n_params = len(in_names)
    all_names = in_names + out_names + ([pid] if pid else [])

    def _jbody(*args):
        operands = list(args)
        if pid is not None:
            operands.append(bass2jax.partition_id_tensor())
        return tuple(
            _bass_exec_p.bind(
                *operands,
                out_avals=tuple(out_avals),
                in_names=tuple(all_names),
                out_names=tuple(out_names),
                lowering_input_output_aliases=(),
                sim_require_finite=True,
                sim_require_nnan=True,
                nc=nc,
            )
        )

    devices = jax.devices()[:N_CORES]
    mesh = Mesh(np.asarray(devices), ("core",))
    specs = (PartitionSpec("core"),) * (n_params + len(out_names))
    out_specs = (PartitionSpec("core"),) * len(out_names)
    fn = jax.jit(
        shard_map(_jbody, mesh=mesh, in_specs=specs, out_specs=out_specs,
                  check_rep=False),
        keep_unused=True,
    )
    concat_zero = [
        np.zeros((N_CORES * z.shape[0], *z.shape[1:]), z.dtype) for z in zero_outs
    ]
    _CACHE["runner"] = (fn, in_names, out_avals, concat_zero)
    return _CACHE["runner"]


def kernel(x, W_qkv, b_qkv, W_proj, b_proj):
    in_maps = _shard_inputs(x, W_qkv, b_qkv, W_proj, b_proj)
    try:
        fn, in_names, out_avals, concat_zero = _get_runner()
        concat_in = [
            np.concatenate([in_maps[c][n] for c in range(N_CORES)], axis=0)
            for n in in_names
        ]
        outs = fn(*concat_in, *concat_zero)
        arr = np.asarray(outs[0]).reshape(N_CORES, L, D)
        out = np.zeros((B, L, D), dtype=np.float32)
        for c in range(N_CORES):
            out[c // 4] += arr[c]
        return out
    except Exception:
        nc = get_nc()
        res = run_bass_kernel_spmd(nc, in_maps, core_ids=list(range(N_CORES)))
        out = np.zeros((B, L, D), dtype=np.float32)
        for c in range(N_CORES):
            out[c // 4] += res.results[c]["out"]
        return out
